# revision 42
# baseline (speedup 1.0000x reference)
"""CRF loss kernel for Trainium2 (8 NeuronCores).

Forward CRF recursion in linear space:
    alpha_t = (alpha_{t-1} @ expT) * exp(o_t)
Positive matrices forget initial conditions geometrically (Birkhoff
contraction; here expT entries are within e^{+-0.5} of 1, so the rate is
~5x per step). The sequence is therefore cut into C short chunks; each
chunk's chain is started from a uniform state W steps early (burn-in) so
its state direction converges to the true alpha before the chunk begins.
Per-group log-growth over the chunk is exact, directions are accurate to
~kappa^W; the scalar log-partition is stitched on the host in fp64.

Device layout: state V[128 labels (partitions), G groups (free dim)].
One matmul with the constant expT as stationary weights advances ALL G
groups one step; a DVE elementwise multiply applies the per-position
emission factors (host-gathered tiles). Emissions are pre-scaled by
exp(-mu) (mu = mean per-step log growth) so no renormalisation is ever
needed: fp32 state drift stays within e^{+-3}.

Group 0 has no real burn-in data (clamped), so its chunk is recomputed
exactly on the host in fp64 and its device result discarded.
"""

import os
import numpy as np

SEQ = 131072
L = 126          # real labels; transitions is (L+2, L+2) = (128, 128)
NL = 128
N_CORES = 8

# ---- tunables -------------------------------------------------------------
S = 4            # superchains (PSUM banks) per core
G = 512          # groups per superchain (one matmul/DVE op advances G chains)
W = 2            # burn-in steps
DMA_BATCH = 5    # steps loaded per emission DMA
# eviction-path schedule, indexed by global step id: A = DVE multiply from
# PSUM (1x); B = ACT copy to SBUF + DVE bf16 multiply (2x); C = ACT copy +
# GpSimd multiply. Spreads the per-step elementwise work over 3 engines.
PATTERN = "ABCABCABAB"
E_DT = "bfloat16"  # emission/state dtype on device: "float32" or "bfloat16"
# ---------------------------------------------------------------------------

C = N_CORES * S * G               # total groups
CHUNK = SEQ // C                  # steps per group
T = W + CHUNK                     # device steps per superchain
assert CHUNK * C == SEQ

last_exec_time_ns = None


SC_G = [512, 512, 1024]   # superchain widths; sc2 spans two PSUM banks
SC_OFF = [0, 512, 1024]   # group offset of each superchain within a core


def _build_program_raw():
    """Hand-scheduled Bass program (no Tile). Superchains: sc0 = A path
    (DVE multiplies straight from PSUM), sc1 = B path (ACT evicts, DVE
    multiplies bf16 at 2x), sc2 = B path at double width G=1024 (two
    matmuls per step into one 2-bank PSUM tensor, one wide evict, one
    wide multiply) to amortize per-op fixed costs. Semaphore counts are
    real counters, so dependencies propagate transitively.
    """
    from contextlib import ExitStack
    import concourse.bacc as bacc
    import concourse.mybir as mybir

    e_dt = getattr(mybir.dt, E_DT)
    f32 = mybir.dt.float32

    nc = bacc.Bacc("TRN2", target_bir_lowering=False, debug=False)
    w_d = nc.dram_tensor("w", [NL, NL], e_dt, kind="ExternalInput")
    e_ds = [nc.dram_tensor(f"e{s}", [NL, T * g], e_dt, kind="ExternalInput")
            for s, g in enumerate(SC_G)]
    snap_ds = [nc.dram_tensor(f"snap{s}", [NL, g], e_dt, kind="ExternalOutput")
               for s, g in enumerate(SC_G)]
    fin_ds = [nc.dram_tensor(f"fin{s}", [NL, g], e_dt, kind="ExternalOutput")
              for s, g in enumerate(SC_G)]
    NSC = len(SC_G)

    # PE order per step: sc0, sc1, sc2-lo, sc2-hi  (4 matmuls, t >= 1)
    def cnt_mm_last(t, s):      # count after the LAST matmul of (t, s)
        return (t - 1) * 4 + (2 if s < 2 else 4) + (s if s < 2 else 0)

    def cnt_mm(t, s):
        return (t - 1) * 4 + {0: 1, 1: 2, 2: 4}[s]

    def cnt_dve(t, j):          # t=0 entries are the init copies
        return t * 3 + j + 1

    def cnt_act(t, k):          # k: 0 = evict sc1, 1 = evict sc2; t >= 1
        return (t - 1) * 2 + k + 1

    def q_of(t):
        return min(3, ((t + 1) * 4 - 1) // T)  # same boundaries for all g

    with ExitStack() as ctx:
        sems = {n: ctx.enter_context(nc.semaphore(n))
                for n in ("dma", "pe", "dve", "act", "out")}
        for q in range(4):
            for s in range(NSC):
                n = f"eq{q}_{s}"
                sems[n] = ctx.enter_context(nc.semaphore(n))
        w_t = ctx.enter_context(nc.sbuf_tensor("wt", [NL, NL], e_dt))
        e_b = [ctx.enter_context(nc.sbuf_tensor(f"eb{s}", [NL, T * g], e_dt))
               for s, g in enumerate(SC_G)]
        v_b = [[ctx.enter_context(
                    nc.sbuf_tensor(f"v{s}_{b}", [NL, g], e_dt))
                for b in range(2)] for s, g in enumerate(SC_G)]
        x_b = [[ctx.enter_context(
                    nc.sbuf_tensor(f"x{s}_{b}", [NL, SC_G[s]], e_dt))
                for b in range(2)] for s in (1, 2)]
        ps_b = [[ctx.enter_context(
                    nc.psum_tensor(f"ps{s}_{b}", [NL, g], f32))
                 for b in range(2)] for s, g in enumerate(SC_G)]

        class Waiter:
            def __init__(self, eng):
                self.eng = eng
                self.seen = {}

            def __call__(self, sem_name, val):
                if self.seen.get(sem_name, -1) >= val:
                    return
                self.seen[sem_name] = val
                self.eng.wait_ge(sems[sem_name], val)

        with nc.Block() as block:

            @block.sync
            def _(sync):
                wt = Waiter(sync)
                sync.dma_start(w_t[:], w_d[:]).then_inc(sems["dma"], 16)
                for q in range(4):
                    for s, g in enumerate(SC_G):
                        qc = T * g // 4
                        sl = slice(q * qc, (q + 1) * qc)
                        sync.dma_start(e_b[s][:, sl], e_ds[s][:, sl]) \
                            .then_inc(sems[f"eq{q}_{s}"], 16)
                for s in range(NSC):
                    wt("dve", cnt_dve(W - 1, s))
                    sync.dma_start(snap_ds[s][:], v_b[s][(W - 1) % 2][:]) \
                        .then_inc(sems["out"], 16)
                for s in range(NSC):
                    wt("dve", cnt_dve(T - 1, s))
                    sync.dma_start(fin_ds[s][:], v_b[s][(T - 1) % 2][:]) \
                        .then_inc(sems["out"], 16)
                sync.wait_ge(sems["out"], 16 * 2 * NSC)

            @block.tensor
            def _(tensor):
                wt = Waiter(tensor)
                wt("dma", 16)  # weights resident
                for t in range(1, T):
                    for s in range(NSC):
                        wt("dve", cnt_dve(t - 1, s))
                        if s < 2:
                            tensor.matmul(
                                ps_b[s][t % 2][:], w_t[:],
                                v_b[s][(t - 1) % 2][:],
                                start=True, stop=True,
                            ).then_inc(sems["pe"], 1)
                        else:
                            for h in range(2):
                                hs = slice(h * 512, (h + 1) * 512)
                                tensor.matmul(
                                    ps_b[2][t % 2][:, hs], w_t[:],
                                    v_b[2][(t - 1) % 2][:, hs],
                                    start=True, stop=True,
                                ).then_inc(sems["pe"], 1)

            @block.vector
            def _(vector):
                wt = Waiter(vector)
                for s, g in enumerate(SC_G):
                    wt(f"eq0_{s}", 16)
                    vector.tensor_copy(
                        v_b[s][0][:], e_b[s][:, 0:g]).then_inc(sems["dve"], 1)
                for t in range(1, T):
                    if t == W + 1:
                        wt("out", 16 * NSC)  # snapshots shipped before reuse
                    for s, g in enumerate(SC_G):
                        wt(f"eq{q_of(t)}_{s}", 16)
                        if s >= 1:
                            wt("act", cnt_act(t, s - 1))
                            in0 = x_b[s - 1][t % 2][:]
                        else:
                            wt("pe", cnt_mm(t, s))
                            in0 = ps_b[s][t % 2][:]
                        vector.tensor_tensor(
                            v_b[s][t % 2][:], in0,
                            e_b[s][:, t * g:(t + 1) * g],
                            op=mybir.AluOpType.mult,
                        ).then_inc(sems["dve"], 1)

            @block.scalar
            def _(scalar):
                wt = Waiter(scalar)
                for t in range(1, T):
                    for k, s in ((0, 1), (1, 2)):
                        wt("pe", cnt_mm(t, s))
                        if t >= 3:
                            wt("dve", cnt_dve(t - 2, s))
                        scalar.activation(
                            x_b[s - 1][t % 2][:], ps_b[s][t % 2][:],
                            mybir.ActivationFunctionType.Copy,
                        ).then_inc(sems["act"], 1)

            @block.gpsimd
            def _(gpsimd):
                pass

        nc.compile()
        nc.finalize()
    return nc


def _build_program():
    import concourse.bacc as bacc
    import concourse.mybir as mybir
    from concourse.tile import TileContext

    e_dt = getattr(mybir.dt, E_DT)
    f32 = mybir.dt.float32

    nc = bacc.Bacc("TRN2", target_bir_lowering=False, debug=False)
    w_d = nc.dram_tensor("w", [NL, NL], e_dt, kind="ExternalInput")
    e_ds = [
        nc.dram_tensor(f"e{s}", [NL, T * G], e_dt, kind="ExternalInput")
        for s in range(S)
    ]
    snap_ds = [
        nc.dram_tensor(f"snap{s}", [NL, G], e_dt, kind="ExternalOutput")
        for s in range(S)
    ]
    fin_ds = [
        nc.dram_tensor(f"fin{s}", [NL, G], e_dt, kind="ExternalOutput")
        for s in range(S)
    ]

    with TileContext(nc) as tc:
        with tc.tile_pool(name="wp", bufs=1) as wp, \
             tc.tile_pool(name="ep", bufs=2) as ep, \
             tc.tile_pool(name="vp", bufs=3) as vp, \
             tc.tile_pool(name="xp", bufs=2) as xp, \
             tc.tile_pool(name="pp", bufs=2, space="PSUM") as pp:
            # Stage weights through a DVE copy: the first matmul then waits
            # only on the DVE semaphore (PE LDW allows a single sync wait).
            w_stage = wp.tile([NL, NL], e_dt, tag="wstage")
            nc.sync.dma_start(w_stage[:], w_d[:])
            w_t = wp.tile([NL, NL], e_dt, tag="wt")
            nc.vector.tensor_copy(w_t[:], w_stage[:])
            v_cur = []
            for s in range(S):
                v0 = vp.tile([NL, G], e_dt, tag=f"v{s}")
                nc.vector.memset(v0[:], 1.0)
                v_cur.append(v0)
            assert T % DMA_BATCH == 0
            e_quad = [None] * S
            for tq in range(T // DMA_BATCH):
                for s in range(S):
                    eq = ep.tile([NL, DMA_BATCH * G], e_dt, tag=f"e{s}")
                    nc.sync.dma_start(
                        eq[:],
                        e_ds[s][:, tq * DMA_BATCH * G:(tq + 1) * DMA_BATCH * G],
                    )
                    e_quad[s] = eq
                for dt in range(DMA_BATCH):
                    t = tq * DMA_BATCH + dt
                    for s in range(S):
                        ps = pp.tile([NL, G], f32, tag=f"ps{s}")
                        nc.tensor.matmul(ps[:], w_t[:], v_cur[s][:],
                                         start=True, stop=True)
                        e_ap = e_quad[s][:, dt * G:(dt + 1) * G]
                        path = PATTERN[(t * S + s) % len(PATTERN)]
                        v_n = vp.tile([NL, G], e_dt, tag=f"v{s}")
                        if path == "A":
                            nc.vector.tensor_tensor(
                                v_n[:], ps[:], e_ap,
                                op=mybir.AluOpType.mult)
                        else:
                            x = xp.tile([NL, G], e_dt, tag=f"x{s}")
                            nc.scalar.activation(
                                x[:], ps[:],
                                mybir.ActivationFunctionType.Copy)
                            engine = (nc.vector if path == "B"
                                      else nc.gpsimd)
                            engine.tensor_tensor(
                                v_n[:], x[:], e_ap,
                                op=mybir.AluOpType.mult)
                        v_cur[s] = v_n
                        if t == W - 1:
                            nc.sync.dma_start(snap_ds[s][:], v_n[:])
                        if t == T - 1:
                            nc.sync.dma_start(fin_ds[s][:], v_n[:])
    nc.compile()
    nc.finalize()
    return nc


def _profiled_run(nc, in_maps):
    """Run via PJRT with local NTFF profiling (core 0); returns (res, ns)."""
    import tempfile
    from concourse.bass_utils import run_bass_kernel_spmd
    from trn_agent_boot.trn_boot import _ntff_profile_via_ctypes

    hookf = _ntff_profile_via_ctypes("/opt/axon/libaxon_pjrt.so")
    neff_dir = tempfile.mkdtemp(prefix="crfprof_")
    exec_ns = None
    if hookf is None:
        res = run_bass_kernel_spmd(nc, in_maps, list(range(N_CORES)))
        return res, None
    with hookf(neff_dir, [0]):
        res = run_bass_kernel_spmd(nc, in_maps, list(range(N_CORES)))
    try:
        import gauge.profiler
        from concourse._compat import FishPath

        profile = gauge.profiler.Profile(
            profile_path=FishPath(neff_dir),
            kernel_dev_mode=True,
            profile_on_exit=False,
            bass_kernel=nc.m,
            offline_processing=True,
            fname="*_body*",
        )
        results = profile.to_perfetto(model_index=(0,))
        if results:
            exec_ns = results[0].exec_time_ns
            print(f"[profile] core0 exec {exec_ns} ns, "
                  f"trace: {results[0].trace_path}")
    except Exception as e:  # profiling must never break the run
        import traceback
        traceback.print_exc()
        print(f"[profile] failed: {e!r} (dir {neff_dir})")
    return res, exec_ns


def _estimate_mu(obs_pad, expTz64, n=256, skip=32):
    """Mean per-step log growth of the linear-space recursion (fp64)."""
    u = np.full(NL, 1.0 / NL, np.float64)
    logs = []
    for t in range(skip + n):
        u = (u @ expTz64) * np.exp(obs_pad[t].astype(np.float64))
        ssum = u.sum()
        logs.append(np.log(ssum))
        u /= ssum
    return float(np.mean(logs[skip:]))


def kernel(pred: np.ndarray, transitions: np.ndarray, ref: np.ndarray) -> np.ndarray:
    global last_exec_time_ns
    pred = np.asarray(pred)
    transitions = np.asarray(transitions)
    ref = np.asarray(ref)
    assert pred.shape == (SEQ, L)

    T64 = transitions.astype(np.float64)
    expTz64 = np.exp(T64)
    expTz64[:, L:] = 0.0            # dummy labels carry no mass mid-sequence

    obs_pad = np.zeros((SEQ, NL), np.float32)
    obs_pad[:, :L] = pred

    mu = _estimate_mu(obs_pad, expTz64)

    # E128T[l, p] = exp(obs[p, l] - mu), transposed, with W left-pad columns
    # replicating position 0 (burn-in clamp for the first groups).
    E128 = np.exp(obs_pad - np.float32(mu))
    E128T = np.ascontiguousarray(E128.T)                  # [128, SEQ]
    EPAD = np.concatenate(
        [np.repeat(E128T[:, :1], W, axis=1), E128T], axis=1
    )                                                     # [128, W + SEQ]

    np_e_dt = np.float32
    if E_DT == "bfloat16":
        import ml_dtypes
        np_e_dt = ml_dtypes.bfloat16
        EPAD = EPAD.astype(np_e_dt)

    # Device emission tiles: e[c][s][l, t, g] = EPAD[l, A + t + g*CHUNK],
    # A = (c*S + s)*G*CHUNK  (EPAD coords already include the +W shift).
    itemsize = EPAD.strides[1]
    sc_g = [512, 512, 1024]
    sc_off = [0, 512, 1024]
    e_blocks = []
    for c in range(N_CORES):
        per_s = []
        for g, off in zip(sc_g, sc_off):
            A = (c * 2048 + off) * CHUNK
            blk = np.lib.stride_tricks.as_strided(
                EPAD[:, A:],
                shape=(NL, T, g),
                strides=(EPAD.strides[0], itemsize, CHUNK * itemsize),
            )
            per_s.append(np.ascontiguousarray(blk).reshape(NL, T * g))
        e_blocks.append(per_s)

    wz32 = expTz64.astype(np_e_dt)
    if os.environ.get("CRF_TILE"):
        nc = _build_program()
    else:
        nc = _build_program_raw()

    from concourse.bass_utils import run_bass_kernel_spmd
    in_maps = [
        {"w": wz32, **{f"e{s}": e_blocks[c][s] for s in range(len(sc_g))}}
        for c in range(N_CORES)
    ]
    if os.environ.get("CRF_TRACE"):
        res, last_exec_time_ns = _profiled_run(nc, in_maps)
    else:
        res = run_bass_kernel_spmd(nc, in_maps, list(range(N_CORES)))

    # --- host stitch (fp64) -------------------------------------------------
    snap_sums = np.empty(C, np.float64)
    fin_sums = np.empty(C, np.float64)
    d_last = None
    for c in range(N_CORES):
        for s, (g, off) in enumerate(zip(sc_g, sc_off)):
            j0 = c * 2048 + off
            snap = res.results[c][f"snap{s}"].astype(np.float64)
            fin = res.results[c][f"fin{s}"].astype(np.float64)
            snap_sums[j0:j0 + g] = snap.sum(axis=0)
            fin_sums[j0:j0 + g] = fin.sum(axis=0)
            if c == N_CORES - 1 and s == len(sc_g) - 1:
                d_last = fin[:, g - 1] / fin[:, g - 1].sum()

    # group 0 exactly on host: start from begin-state e_{126}
    u = np.zeros(NL, np.float64)
    u[L] = 1.0
    log_g0 = 0.0
    for t in range(CHUNK):
        u = (u @ expTz64) * np.exp(obs_pad[t].astype(np.float64))
        ssum = u.sum()
        log_g0 += np.log(ssum)
        u /= ssum

    ratios = np.log(fin_sums[1:]) - np.log(snap_sums[1:])
    logZ = (
        log_g0
        + ratios.sum() + (C - 1) * CHUNK * mu
        + np.log(np.dot(d_last, np.exp(T64[:, L + 1])))
    )

    # gold path score
    idx = np.arange(SEQ)
    real = pred.astype(np.float64)[idx, ref].sum()
    padded = np.concatenate(
        [np.array([L], ref.dtype), ref, np.array([L + 1], ref.dtype)]
    )
    real += T64[padded[:-1], padded[1:]].sum()

    return np.float32(logZ - real)


# revision 43
# speedup vs baseline: 1.1846x; 1.1846x over previous
"""CRF loss kernel for Trainium2 (8 NeuronCores).

Forward CRF recursion in linear space:
    alpha_t = (alpha_{t-1} @ expT) * exp(o_t)
Positive matrices forget initial conditions geometrically (Birkhoff
contraction; here expT entries are within e^{+-0.5} of 1, so the rate is
~5x per step). The sequence is therefore cut into C short chunks; each
chunk's chain is started from a uniform state W steps early (burn-in) so
its state direction converges to the true alpha before the chunk begins.
Per-group log-growth over the chunk is exact, directions are accurate to
~kappa^W; the scalar log-partition is stitched on the host in fp64.

Device layout: state V[128 labels (partitions), G groups (free dim)].
One matmul with the constant expT as stationary weights advances ALL G
groups one step; a DVE elementwise multiply applies the per-position
emission factors (host-gathered tiles). Emissions are pre-scaled by
exp(-mu) (mu = mean per-step log growth) so no renormalisation is ever
needed: fp32 state drift stays within e^{+-3}.

Group 0 has no real burn-in data (clamped), so its chunk is recomputed
exactly on the host in fp64 and its device result discarded.
"""

import os
import numpy as np

SEQ = 131072
L = 126          # real labels; transitions is (L+2, L+2) = (128, 128)
NL = 128
N_CORES = 8

# ---- tunables -------------------------------------------------------------
S = 4            # superchains (PSUM banks) per core
G = 512          # groups per superchain (one matmul/DVE op advances G chains)
W = 2            # burn-in steps
DMA_BATCH = 5    # steps loaded per emission DMA
# eviction-path schedule, indexed by global step id: A = DVE multiply from
# PSUM (1x); B = ACT copy to SBUF + DVE bf16 multiply (2x); C = ACT copy +
# GpSimd multiply. Spreads the per-step elementwise work over 3 engines.
PATTERN = "ABCABCABAB"
E_DT = "bfloat16"  # emission/state dtype on device: "float32" or "bfloat16"
# ---------------------------------------------------------------------------

C = N_CORES * S * G               # total groups
CHUNK = SEQ // C                  # steps per group
T = W + CHUNK                     # device steps per superchain
assert CHUNK * C == SEQ

last_exec_time_ns = None


def _build_program_raw():
    """Hand-scheduled Bass program (no Tile): per-superchain fixed paths
    s0,s1 = A (DVE multiplies straight from PSUM), s2 = B (ACT evicts to
    SBUF, DVE multiplies at 2x), s3 = C (ACT evicts, GpSimd multiplies,
    with an explicit drain before the semaphore increment for cross-engine
    write visibility). Semaphore counts are real counters, so dependencies
    propagate transitively; each op needs at most two standalone waits.
    """
    from contextlib import ExitStack
    import concourse.bacc as bacc
    import concourse.mybir as mybir

    e_dt = getattr(mybir.dt, E_DT)
    f32 = mybir.dt.float32
    assert S == 4 and W >= 2

    nc = bacc.Bacc("TRN2", target_bir_lowering=False, debug=False)
    w_d = nc.dram_tensor("w", [NL, NL], e_dt, kind="ExternalInput")
    e_ds = [nc.dram_tensor(f"e{s}", [NL, T * G], e_dt, kind="ExternalInput")
            for s in range(S)]
    snap_ds = [nc.dram_tensor(f"snap{s}", [NL, G], e_dt, kind="ExternalOutput")
               for s in range(S)]
    fin_ds = [nc.dram_tensor(f"fin{s}", [NL, G], e_dt, kind="ExternalOutput")
              for s in range(S)]

    def cnt_mm(t, s):           # matmuls run for t >= 1
        return (t - 1) * S + s + 1

    def cnt_dve(t, j):          # j = s; t=0 ops are the init copies
        return t * 4 + j + 1

    def cnt_act(t, k):          # k: 0,1,2 = evict s1,s2,s3; t >= 1
        return (t - 1) * 3 + k + 1

    QCOLS = T * G // 4
    def q_of(t):
        return min(3, ((t + 1) * G - 1) // QCOLS)

    prod_sem_of = {0: "dve", 1: "dve", 2: "dve", 3: "dve"}

    def cnt_prod(t, s):
        return cnt_dve(t, s)

    with ExitStack() as ctx:
        sems = {n: ctx.enter_context(nc.semaphore(n))
                for n in ("dma", "pe", "dve", "act", "gp", "out")}
        for q in range(4):
            for s in range(S):
                n = f"eq{q}_{s}"
                sems[n] = ctx.enter_context(nc.semaphore(n))
        w_t = ctx.enter_context(nc.sbuf_tensor("wt", [NL, NL], e_dt))
        e_b = [ctx.enter_context(nc.sbuf_tensor(f"eb{s}", [NL, T * G], e_dt))
               for s in range(S)]
        v_b = [[ctx.enter_context(
                    nc.sbuf_tensor(f"v{s}_{b}", [NL, G], e_dt))
                for b in range(2)] for s in range(S)]
        x_b = [[ctx.enter_context(
                    nc.sbuf_tensor(f"x{s}_{b}", [NL, G], e_dt))
                for b in range(2)] for s in (1, 2, 3)]
        ps_b = [[ctx.enter_context(
                    nc.psum_tensor(f"ps{s}_{b}", [NL, G], f32))
                 for b in range(2)] for s in range(S)]

        class Waiter:
            """Emit a wait only if it raises this engine's known floor."""
            def __init__(self, eng):
                self.eng = eng
                self.seen = {}

            def __call__(self, sem_name, val):
                if self.seen.get(sem_name, -1) >= val:
                    return
                self.seen[sem_name] = val
                self.eng.wait_ge(sems[sem_name], val)

        with nc.Block() as block:

            @block.sync
            def _(sync):
                wt = Waiter(sync)
                sync.dma_start(w_t[:], w_d[:]).then_inc(sems["dma"], 16)
                for q in range(4):
                    for s in range(S):
                        sl = slice(q * QCOLS, (q + 1) * QCOLS)
                        sync.dma_start(e_b[s][:, sl], e_ds[s][:, sl]) \
                            .then_inc(sems[f"eq{q}_{s}"], 16)
                for s in range(S):
                    wt(prod_sem_of[s], cnt_prod(W - 1, s))
                    sync.dma_start(snap_ds[s][:], v_b[s][(W - 1) % 2][:]) \
                        .then_inc(sems["out"], 16)
                for s in range(S):
                    wt(prod_sem_of[s], cnt_prod(T - 1, s))
                    sync.dma_start(fin_ds[s][:], v_b[s][(T - 1) % 2][:]) \
                        .then_inc(sems["out"], 16)
                sync.wait_ge(sems["out"], 16 * 2 * S)

            @block.tensor
            def _(tensor):
                wt = Waiter(tensor)
                wt("dma", 16)  # weights resident
                for t in range(1, T):
                    for s in range(S):
                        wt(prod_sem_of[s], cnt_prod(t - 1, s))
                        tensor.matmul(
                            ps_b[s][t % 2][:], w_t[:],
                            v_b[s][(t - 1) % 2][:],
                            start=True, stop=True,
                        ).then_inc(sems["pe"], 1)

            @block.vector
            def _(vector):
                wt = Waiter(vector)
                for s in range(S):
                    wt(f"eq0_{s}", 16)
                    vector.tensor_copy(
                        v_b[s][0][:], e_b[s][:, 0:G]).then_inc(sems["dve"], 1)
                for t in range(1, T):
                    if t == W + 1:
                        wt("out", 16 * S)  # snapshots shipped before reuse
                    for s in range(S):
                        wt(f"eq{q_of(t)}_{s}", 16)
                        if s >= 1:
                            wt("act", cnt_act(t, s - 1))
                            in0 = x_b[s - 1][t % 2][:]
                        else:
                            wt("pe", cnt_mm(t, s))
                            in0 = ps_b[s][t % 2][:]
                        vector.tensor_tensor(
                            v_b[s][t % 2][:], in0,
                            e_b[s][:, t * G:(t + 1) * G],
                            op=mybir.AluOpType.mult,
                        ).then_inc(sems["dve"], 1)

            @block.scalar
            def _(scalar):
                wt = Waiter(scalar)
                for t in range(1, T):
                    for k, s in ((0, 1), (1, 2), (2, 3)):
                        wt("pe", cnt_mm(t, s))
                        if t >= 3:
                            wt("dve", cnt_dve(t - 2, s))
                        scalar.activation(
                            x_b[s - 1][t % 2][:], ps_b[s][t % 2][:],
                            mybir.ActivationFunctionType.Copy,
                        ).then_inc(sems["act"], 1)

            @block.gpsimd
            def _(gpsimd):
                pass

        nc.compile()
        nc.finalize()
    return nc


def _build_program():
    import concourse.bacc as bacc
    import concourse.mybir as mybir
    from concourse.tile import TileContext

    e_dt = getattr(mybir.dt, E_DT)
    f32 = mybir.dt.float32

    nc = bacc.Bacc("TRN2", target_bir_lowering=False, debug=False)
    w_d = nc.dram_tensor("w", [NL, NL], e_dt, kind="ExternalInput")
    e_ds = [
        nc.dram_tensor(f"e{s}", [NL, T * G], e_dt, kind="ExternalInput")
        for s in range(S)
    ]
    snap_ds = [
        nc.dram_tensor(f"snap{s}", [NL, G], e_dt, kind="ExternalOutput")
        for s in range(S)
    ]
    fin_ds = [
        nc.dram_tensor(f"fin{s}", [NL, G], e_dt, kind="ExternalOutput")
        for s in range(S)
    ]

    with TileContext(nc) as tc:
        with tc.tile_pool(name="wp", bufs=1) as wp, \
             tc.tile_pool(name="ep", bufs=2) as ep, \
             tc.tile_pool(name="vp", bufs=3) as vp, \
             tc.tile_pool(name="xp", bufs=2) as xp, \
             tc.tile_pool(name="pp", bufs=2, space="PSUM") as pp:
            # Stage weights through a DVE copy: the first matmul then waits
            # only on the DVE semaphore (PE LDW allows a single sync wait).
            w_stage = wp.tile([NL, NL], e_dt, tag="wstage")
            nc.sync.dma_start(w_stage[:], w_d[:])
            w_t = wp.tile([NL, NL], e_dt, tag="wt")
            nc.vector.tensor_copy(w_t[:], w_stage[:])
            v_cur = []
            for s in range(S):
                v0 = vp.tile([NL, G], e_dt, tag=f"v{s}")
                nc.vector.memset(v0[:], 1.0)
                v_cur.append(v0)
            assert T % DMA_BATCH == 0
            e_quad = [None] * S
            for tq in range(T // DMA_BATCH):
                for s in range(S):
                    eq = ep.tile([NL, DMA_BATCH * G], e_dt, tag=f"e{s}")
                    nc.sync.dma_start(
                        eq[:],
                        e_ds[s][:, tq * DMA_BATCH * G:(tq + 1) * DMA_BATCH * G],
                    )
                    e_quad[s] = eq
                for dt in range(DMA_BATCH):
                    t = tq * DMA_BATCH + dt
                    for s in range(S):
                        ps = pp.tile([NL, G], f32, tag=f"ps{s}")
                        nc.tensor.matmul(ps[:], w_t[:], v_cur[s][:],
                                         start=True, stop=True)
                        e_ap = e_quad[s][:, dt * G:(dt + 1) * G]
                        path = PATTERN[(t * S + s) % len(PATTERN)]
                        v_n = vp.tile([NL, G], e_dt, tag=f"v{s}")
                        if path == "A":
                            nc.vector.tensor_tensor(
                                v_n[:], ps[:], e_ap,
                                op=mybir.AluOpType.mult)
                        else:
                            x = xp.tile([NL, G], e_dt, tag=f"x{s}")
                            nc.scalar.activation(
                                x[:], ps[:],
                                mybir.ActivationFunctionType.Copy)
                            engine = (nc.vector if path == "B"
                                      else nc.gpsimd)
                            engine.tensor_tensor(
                                v_n[:], x[:], e_ap,
                                op=mybir.AluOpType.mult)
                        v_cur[s] = v_n
                        if t == W - 1:
                            nc.sync.dma_start(snap_ds[s][:], v_n[:])
                        if t == T - 1:
                            nc.sync.dma_start(fin_ds[s][:], v_n[:])
    nc.compile()
    nc.finalize()
    return nc


def _profiled_run(nc, in_maps):
    """Run via PJRT with local NTFF profiling (core 0); returns (res, ns)."""
    import tempfile
    from concourse.bass_utils import run_bass_kernel_spmd
    from trn_agent_boot.trn_boot import _ntff_profile_via_ctypes

    hookf = _ntff_profile_via_ctypes("/opt/axon/libaxon_pjrt.so")
    neff_dir = tempfile.mkdtemp(prefix="crfprof_")
    exec_ns = None
    if hookf is None:
        res = run_bass_kernel_spmd(nc, in_maps, list(range(N_CORES)))
        return res, None
    with hookf(neff_dir, [0]):
        res = run_bass_kernel_spmd(nc, in_maps, list(range(N_CORES)))
    try:
        import gauge.profiler
        from concourse._compat import FishPath

        profile = gauge.profiler.Profile(
            profile_path=FishPath(neff_dir),
            kernel_dev_mode=True,
            profile_on_exit=False,
            bass_kernel=nc.m,
            offline_processing=True,
            fname="*_body*",
        )
        results = profile.to_perfetto(model_index=(0,))
        if results:
            exec_ns = results[0].exec_time_ns
            print(f"[profile] core0 exec {exec_ns} ns, "
                  f"trace: {results[0].trace_path}")
    except Exception as e:  # profiling must never break the run
        import traceback
        traceback.print_exc()
        print(f"[profile] failed: {e!r} (dir {neff_dir})")
    return res, exec_ns


def _estimate_mu(obs_pad, expTz64, n=256, skip=32):
    """Mean per-step log growth of the linear-space recursion (fp64)."""
    u = np.full(NL, 1.0 / NL, np.float64)
    logs = []
    for t in range(skip + n):
        u = (u @ expTz64) * np.exp(obs_pad[t].astype(np.float64))
        ssum = u.sum()
        logs.append(np.log(ssum))
        u /= ssum
    return float(np.mean(logs[skip:]))


def kernel(pred: np.ndarray, transitions: np.ndarray, ref: np.ndarray) -> np.ndarray:
    global last_exec_time_ns
    pred = np.asarray(pred)
    transitions = np.asarray(transitions)
    ref = np.asarray(ref)
    assert pred.shape == (SEQ, L)

    T64 = transitions.astype(np.float64)
    expTz64 = np.exp(T64)
    expTz64[:, L:] = 0.0            # dummy labels carry no mass mid-sequence

    obs_pad = np.zeros((SEQ, NL), np.float32)
    obs_pad[:, :L] = pred

    mu = _estimate_mu(obs_pad, expTz64)

    # E128T[l, p] = exp(obs[p, l] - mu), transposed, with W left-pad columns
    # replicating position 0 (burn-in clamp for the first groups).
    E128 = np.exp(obs_pad - np.float32(mu))
    E128T = np.ascontiguousarray(E128.T)                  # [128, SEQ]
    EPAD = np.concatenate(
        [np.repeat(E128T[:, :1], W, axis=1), E128T], axis=1
    )                                                     # [128, W + SEQ]

    np_e_dt = np.float32
    if E_DT == "bfloat16":
        import ml_dtypes
        np_e_dt = ml_dtypes.bfloat16
        EPAD = EPAD.astype(np_e_dt)

    # Device emission tiles: e[c][s][l, t, g] = EPAD[l, A + t + g*CHUNK],
    # A = (c*S + s)*G*CHUNK  (EPAD coords already include the +W shift).
    itemsize = EPAD.strides[1]
    e_blocks = []
    for c in range(N_CORES):
        per_s = []
        for s in range(S):
            A = (c * S + s) * G * CHUNK
            blk = np.lib.stride_tricks.as_strided(
                EPAD[:, A:],
                shape=(NL, T, G),
                strides=(EPAD.strides[0], itemsize, CHUNK * itemsize),
            )
            per_s.append(np.ascontiguousarray(blk).reshape(NL, T * G))
        e_blocks.append(per_s)

    wz32 = expTz64.astype(np_e_dt)
    if os.environ.get("CRF_TILE"):
        nc = _build_program()
    else:
        nc = _build_program_raw()

    from concourse.bass_utils import run_bass_kernel_spmd
    in_maps = [
        {"w": wz32, **{f"e{s}": e_blocks[c][s] for s in range(S)}}
        for c in range(N_CORES)
    ]
    if os.environ.get("CRF_TRACE"):
        res, last_exec_time_ns = _profiled_run(nc, in_maps)
    else:
        res = run_bass_kernel_spmd(nc, in_maps, list(range(N_CORES)))

    # --- host stitch (fp64) -------------------------------------------------
    snap_sums = np.empty(C, np.float64)
    fin_sums = np.empty(C, np.float64)
    d_last = None
    for c in range(N_CORES):
        for s in range(S):
            j0 = (c * S + s) * G
            snap = res.results[c][f"snap{s}"].astype(np.float64)
            fin = res.results[c][f"fin{s}"].astype(np.float64)
            snap_sums[j0:j0 + G] = snap.sum(axis=0)
            fin_sums[j0:j0 + G] = fin.sum(axis=0)
            if c == N_CORES - 1 and s == S - 1:
                d_last = fin[:, G - 1] / fin[:, G - 1].sum()

    # group 0 exactly on host: start from begin-state e_{126}
    u = np.zeros(NL, np.float64)
    u[L] = 1.0
    log_g0 = 0.0
    for t in range(CHUNK):
        u = (u @ expTz64) * np.exp(obs_pad[t].astype(np.float64))
        ssum = u.sum()
        log_g0 += np.log(ssum)
        u /= ssum

    ratios = np.log(fin_sums[1:]) - np.log(snap_sums[1:])
    logZ = (
        log_g0
        + ratios.sum() + (C - 1) * CHUNK * mu
        + np.log(np.dot(d_last, np.exp(T64[:, L + 1])))
    )

    # gold path score
    idx = np.arange(SEQ)
    real = pred.astype(np.float64)[idx, ref].sum()
    padded = np.concatenate(
        [np.array([L], ref.dtype), ref, np.array([L + 1], ref.dtype)]
    )
    real += T64[padded[:-1], padded[1:]].sum()

    return np.float32(logZ - real)


# revision 44
# speedup vs baseline: 1.2618x; 1.0652x over previous
"""CRF loss kernel for Trainium2 (8 NeuronCores).

Forward CRF recursion in linear space:
    alpha_t = (alpha_{t-1} @ expT) * exp(o_t)
Positive matrices forget initial conditions geometrically (Birkhoff
contraction; here expT entries are within e^{+-0.5} of 1, so the rate is
~5x per step). The sequence is therefore cut into C short chunks; each
chunk's chain is started from a uniform state W steps early (burn-in) so
its state direction converges to the true alpha before the chunk begins.
Per-group log-growth over the chunk is exact, directions are accurate to
~kappa^W; the scalar log-partition is stitched on the host in fp64.

Device layout: state V[128 labels (partitions), G groups (free dim)].
One matmul with the constant expT as stationary weights advances ALL G
groups one step; a DVE elementwise multiply applies the per-position
emission factors (host-gathered tiles). Emissions are pre-scaled by
exp(-mu) (mu = mean per-step log growth) so no renormalisation is ever
needed: fp32 state drift stays within e^{+-3}.

Group 0 has no real burn-in data (clamped), so its chunk is recomputed
exactly on the host in fp64 and its device result discarded.
"""

import os
import numpy as np

SEQ = 131072
L = 126          # real labels; transitions is (L+2, L+2) = (128, 128)
NL = 128
N_CORES = 8

# ---- tunables -------------------------------------------------------------
S = 4            # superchains (PSUM banks) per core
G = 512          # groups per superchain (one matmul/DVE op advances G chains)
W = 2            # burn-in steps
DMA_BATCH = 5    # steps loaded per emission DMA
# eviction-path schedule, indexed by global step id: A = DVE multiply from
# PSUM (1x); B = ACT copy to SBUF + DVE bf16 multiply (2x); C = ACT copy +
# GpSimd multiply. Spreads the per-step elementwise work over 3 engines.
PATTERN = "ABCABCABAB"
E_DT = "bfloat16"  # emission/state dtype on device: "float32" or "bfloat16"
# ---------------------------------------------------------------------------

C = N_CORES * S * G               # total groups
CHUNK = SEQ // C                  # steps per group
T = W + CHUNK                     # device steps per superchain
assert CHUNK * C == SEQ

last_exec_time_ns = None


def _build_program_raw():
    """Hand-scheduled Bass program (no Tile): per-superchain fixed paths
    s0,s1 = A (DVE multiplies straight from PSUM), s2 = B (ACT evicts to
    SBUF, DVE multiplies at 2x), s3 = C (ACT evicts, GpSimd multiplies,
    with an explicit drain before the semaphore increment for cross-engine
    write visibility). Semaphore counts are real counters, so dependencies
    propagate transitively; each op needs at most two standalone waits.
    """
    from contextlib import ExitStack
    import concourse.bacc as bacc
    import concourse.mybir as mybir

    e_dt = getattr(mybir.dt, E_DT)
    f32 = mybir.dt.float32
    assert S == 4 and W >= 2

    nc = bacc.Bacc("TRN2", target_bir_lowering=False, debug=False)
    w_d = nc.dram_tensor("w", [NL, NL], e_dt, kind="ExternalInput")
    e_ds = [nc.dram_tensor(f"e{s}", [NL, T * G], e_dt, kind="ExternalInput")
            for s in range(S)]
    snap_ds = [nc.dram_tensor(f"snap{s}", [NL, G], e_dt, kind="ExternalOutput")
               for s in range(S)]
    fin_ds = [nc.dram_tensor(f"fin{s}", [NL, G], e_dt, kind="ExternalOutput")
              for s in range(S)]

    def cnt_mm(t, s):           # matmuls run for t >= 1
        return (t - 1) * S + s + 1

    def cnt_dve(t, j):          # j = s; t=0 ops are the init copies
        return t * 4 + j + 1

    def cnt_act(t, k):          # k: 0,1,2 = evict s1,s2,s3; t >= 1
        return (t - 1) * 3 + k + 1

    HCOLS = T * G // 2
    def h_of(t):
        return 0 if (t + 1) * G <= HCOLS else 1

    prod_sem_of = {0: "dve", 1: "dve", 2: "dve", 3: "dve"}

    def cnt_prod(t, s):
        return cnt_dve(t, s)

    with ExitStack() as ctx:
        sems = {n: ctx.enter_context(nc.semaphore(n))
                for n in ("dma", "pe", "dve", "act", "gp", "out")}
        for h in range(2):
            for s in range(S):
                n = f"eh{h}_{s}"
                sems[n] = ctx.enter_context(nc.semaphore(n))
        w_t = ctx.enter_context(nc.sbuf_tensor("wt", [NL, NL], e_dt))
        e_b = [ctx.enter_context(nc.sbuf_tensor(f"eb{s}", [NL, T * G], e_dt))
               for s in range(S)]
        v_b = [[ctx.enter_context(
                    nc.sbuf_tensor(f"v{s}_{b}", [NL, G], e_dt))
                for b in range(2)] for s in range(S)]
        x_b = [[ctx.enter_context(
                    nc.sbuf_tensor(f"x{s}_{b}", [NL, G], e_dt))
                for b in range(2)] for s in (1, 2, 3)]
        ps_b = [[ctx.enter_context(
                    nc.psum_tensor(f"ps{s}_{b}", [NL, G], f32))
                 for b in range(2)] for s in range(S)]

        class Waiter:
            """Emit a wait only if it raises this engine's known floor."""
            def __init__(self, eng):
                self.eng = eng
                self.seen = {}

            def __call__(self, sem_name, val):
                if self.seen.get(sem_name, -1) >= val:
                    return
                self.seen[sem_name] = val
                self.eng.wait_ge(sems[sem_name], val)

        with nc.Block() as block:

            @block.sync
            def _(sync):
                wt = Waiter(sync)
                sync.dma_start(w_t[:], w_d[:]).then_inc(sems["dma"], 16)
                for h in range(2):
                    for s in (0, 1):
                        sl = slice(h * HCOLS, (h + 1) * HCOLS)
                        sync.dma_start(e_b[s][:, sl], e_ds[s][:, sl]) \
                            .then_inc(sems[f"eh{h}_{s}"], 16)
                for s in range(S):
                    wt(prod_sem_of[s], cnt_prod(W - 1, s))
                    sync.dma_start(snap_ds[s][:], v_b[s][(W - 1) % 2][:]) \
                        .then_inc(sems["out"], 16)
                for s in range(S):
                    wt(prod_sem_of[s], cnt_prod(T - 1, s))
                    sync.dma_start(fin_ds[s][:], v_b[s][(T - 1) % 2][:]) \
                        .then_inc(sems["out"], 16)
                sync.wait_ge(sems["out"], 16 * 2 * S)

            @block.tensor
            def _(tensor):
                wt = Waiter(tensor)
                wt("dma", 16)  # weights resident
                for t in range(1, T):
                    for s in range(S):
                        wt(prod_sem_of[s], cnt_prod(t - 1, s))
                        tensor.matmul(
                            ps_b[s][t % 2][:], w_t[:],
                            v_b[s][(t - 1) % 2][:],
                            start=True, stop=True,
                        ).then_inc(sems["pe"], 1)

            @block.vector
            def _(vector):
                wt = Waiter(vector)
                for s in range(S):
                    wt(f"eh0_{s}", 16)
                    vector.tensor_copy(
                        v_b[s][0][:], e_b[s][:, 0:G]).then_inc(sems["dve"], 1)
                for t in range(1, T):
                    if t == W + 1:
                        wt("out", 16 * S)  # snapshots shipped before reuse
                    for s in range(S):
                        wt(f"eh{h_of(t)}_{s}", 16)
                        if s >= 1:
                            wt("act", cnt_act(t, s - 1))
                            in0 = x_b[s - 1][t % 2][:]
                        else:
                            wt("pe", cnt_mm(t, s))
                            in0 = ps_b[s][t % 2][:]
                        vector.tensor_tensor(
                            v_b[s][t % 2][:], in0,
                            e_b[s][:, t * G:(t + 1) * G],
                            op=mybir.AluOpType.mult,
                        ).then_inc(sems["dve"], 1)

            @block.scalar
            def _(scalar):
                wt = Waiter(scalar)
                for t in range(1, T):
                    for k, s in ((0, 1), (1, 2), (2, 3)):
                        wt("pe", cnt_mm(t, s))
                        if t >= 3:
                            wt("dve", cnt_dve(t - 2, s))
                        scalar.activation(
                            x_b[s - 1][t % 2][:], ps_b[s][t % 2][:],
                            mybir.ActivationFunctionType.Copy,
                        ).then_inc(sems["act"], 1)

            @block.gpsimd
            def _(gpsimd):
                # feed sc2/sc3 emissions from the POOL DMA queue in parallel
                for h in range(2):
                    for s in (2, 3):
                        sl = slice(h * HCOLS, (h + 1) * HCOLS)
                        gpsimd.dma_start(e_b[s][:, sl], e_ds[s][:, sl]) \
                            .then_inc(sems[f"eh{h}_{s}"], 16)

        nc.compile()
        nc.finalize()
    return nc


def _build_program():
    import concourse.bacc as bacc
    import concourse.mybir as mybir
    from concourse.tile import TileContext

    e_dt = getattr(mybir.dt, E_DT)
    f32 = mybir.dt.float32

    nc = bacc.Bacc("TRN2", target_bir_lowering=False, debug=False)
    w_d = nc.dram_tensor("w", [NL, NL], e_dt, kind="ExternalInput")
    e_ds = [
        nc.dram_tensor(f"e{s}", [NL, T * G], e_dt, kind="ExternalInput")
        for s in range(S)
    ]
    snap_ds = [
        nc.dram_tensor(f"snap{s}", [NL, G], e_dt, kind="ExternalOutput")
        for s in range(S)
    ]
    fin_ds = [
        nc.dram_tensor(f"fin{s}", [NL, G], e_dt, kind="ExternalOutput")
        for s in range(S)
    ]

    with TileContext(nc) as tc:
        with tc.tile_pool(name="wp", bufs=1) as wp, \
             tc.tile_pool(name="ep", bufs=2) as ep, \
             tc.tile_pool(name="vp", bufs=3) as vp, \
             tc.tile_pool(name="xp", bufs=2) as xp, \
             tc.tile_pool(name="pp", bufs=2, space="PSUM") as pp:
            # Stage weights through a DVE copy: the first matmul then waits
            # only on the DVE semaphore (PE LDW allows a single sync wait).
            w_stage = wp.tile([NL, NL], e_dt, tag="wstage")
            nc.sync.dma_start(w_stage[:], w_d[:])
            w_t = wp.tile([NL, NL], e_dt, tag="wt")
            nc.vector.tensor_copy(w_t[:], w_stage[:])
            v_cur = []
            for s in range(S):
                v0 = vp.tile([NL, G], e_dt, tag=f"v{s}")
                nc.vector.memset(v0[:], 1.0)
                v_cur.append(v0)
            assert T % DMA_BATCH == 0
            e_quad = [None] * S
            for tq in range(T // DMA_BATCH):
                for s in range(S):
                    eq = ep.tile([NL, DMA_BATCH * G], e_dt, tag=f"e{s}")
                    nc.sync.dma_start(
                        eq[:],
                        e_ds[s][:, tq * DMA_BATCH * G:(tq + 1) * DMA_BATCH * G],
                    )
                    e_quad[s] = eq
                for dt in range(DMA_BATCH):
                    t = tq * DMA_BATCH + dt
                    for s in range(S):
                        ps = pp.tile([NL, G], f32, tag=f"ps{s}")
                        nc.tensor.matmul(ps[:], w_t[:], v_cur[s][:],
                                         start=True, stop=True)
                        e_ap = e_quad[s][:, dt * G:(dt + 1) * G]
                        path = PATTERN[(t * S + s) % len(PATTERN)]
                        v_n = vp.tile([NL, G], e_dt, tag=f"v{s}")
                        if path == "A":
                            nc.vector.tensor_tensor(
                                v_n[:], ps[:], e_ap,
                                op=mybir.AluOpType.mult)
                        else:
                            x = xp.tile([NL, G], e_dt, tag=f"x{s}")
                            nc.scalar.activation(
                                x[:], ps[:],
                                mybir.ActivationFunctionType.Copy)
                            engine = (nc.vector if path == "B"
                                      else nc.gpsimd)
                            engine.tensor_tensor(
                                v_n[:], x[:], e_ap,
                                op=mybir.AluOpType.mult)
                        v_cur[s] = v_n
                        if t == W - 1:
                            nc.sync.dma_start(snap_ds[s][:], v_n[:])
                        if t == T - 1:
                            nc.sync.dma_start(fin_ds[s][:], v_n[:])
    nc.compile()
    nc.finalize()
    return nc


def _profiled_run(nc, in_maps):
    """Run via PJRT with local NTFF profiling (core 0); returns (res, ns)."""
    import tempfile
    from concourse.bass_utils import run_bass_kernel_spmd
    from trn_agent_boot.trn_boot import _ntff_profile_via_ctypes

    hookf = _ntff_profile_via_ctypes("/opt/axon/libaxon_pjrt.so")
    neff_dir = tempfile.mkdtemp(prefix="crfprof_")
    exec_ns = None
    if hookf is None:
        res = run_bass_kernel_spmd(nc, in_maps, list(range(N_CORES)))
        return res, None
    with hookf(neff_dir, [0]):
        res = run_bass_kernel_spmd(nc, in_maps, list(range(N_CORES)))
    try:
        import gauge.profiler
        from concourse._compat import FishPath

        profile = gauge.profiler.Profile(
            profile_path=FishPath(neff_dir),
            kernel_dev_mode=True,
            profile_on_exit=False,
            bass_kernel=nc.m,
            offline_processing=True,
            fname="*_body*",
        )
        results = profile.to_perfetto(model_index=(0,))
        if results:
            exec_ns = results[0].exec_time_ns
            print(f"[profile] core0 exec {exec_ns} ns, "
                  f"trace: {results[0].trace_path}")
    except Exception as e:  # profiling must never break the run
        import traceback
        traceback.print_exc()
        print(f"[profile] failed: {e!r} (dir {neff_dir})")
    return res, exec_ns


def _estimate_mu(obs_pad, expTz64, n=256, skip=32):
    """Mean per-step log growth of the linear-space recursion (fp64)."""
    u = np.full(NL, 1.0 / NL, np.float64)
    logs = []
    for t in range(skip + n):
        u = (u @ expTz64) * np.exp(obs_pad[t].astype(np.float64))
        ssum = u.sum()
        logs.append(np.log(ssum))
        u /= ssum
    return float(np.mean(logs[skip:]))


def kernel(pred: np.ndarray, transitions: np.ndarray, ref: np.ndarray) -> np.ndarray:
    global last_exec_time_ns
    pred = np.asarray(pred)
    transitions = np.asarray(transitions)
    ref = np.asarray(ref)
    assert pred.shape == (SEQ, L)

    T64 = transitions.astype(np.float64)
    expTz64 = np.exp(T64)
    expTz64[:, L:] = 0.0            # dummy labels carry no mass mid-sequence

    obs_pad = np.zeros((SEQ, NL), np.float32)
    obs_pad[:, :L] = pred

    mu = _estimate_mu(obs_pad, expTz64)

    # E128T[l, p] = exp(obs[p, l] - mu), transposed, with W left-pad columns
    # replicating position 0 (burn-in clamp for the first groups).
    E128 = np.exp(obs_pad - np.float32(mu))
    E128T = np.ascontiguousarray(E128.T)                  # [128, SEQ]
    EPAD = np.concatenate(
        [np.repeat(E128T[:, :1], W, axis=1), E128T], axis=1
    )                                                     # [128, W + SEQ]

    np_e_dt = np.float32
    if E_DT == "bfloat16":
        import ml_dtypes
        np_e_dt = ml_dtypes.bfloat16
        EPAD = EPAD.astype(np_e_dt)

    # Device emission tiles: e[c][s][l, t, g] = EPAD[l, A + t + g*CHUNK],
    # A = (c*S + s)*G*CHUNK  (EPAD coords already include the +W shift).
    itemsize = EPAD.strides[1]
    e_blocks = []
    for c in range(N_CORES):
        per_s = []
        for s in range(S):
            A = (c * S + s) * G * CHUNK
            blk = np.lib.stride_tricks.as_strided(
                EPAD[:, A:],
                shape=(NL, T, G),
                strides=(EPAD.strides[0], itemsize, CHUNK * itemsize),
            )
            per_s.append(np.ascontiguousarray(blk).reshape(NL, T * G))
        e_blocks.append(per_s)

    wz32 = expTz64.astype(np_e_dt)
    if os.environ.get("CRF_TILE"):
        nc = _build_program()
    else:
        nc = _build_program_raw()

    from concourse.bass_utils import run_bass_kernel_spmd
    in_maps = [
        {"w": wz32, **{f"e{s}": e_blocks[c][s] for s in range(S)}}
        for c in range(N_CORES)
    ]
    if os.environ.get("CRF_TRACE"):
        res, last_exec_time_ns = _profiled_run(nc, in_maps)
    else:
        res = run_bass_kernel_spmd(nc, in_maps, list(range(N_CORES)))

    # --- host stitch (fp64) -------------------------------------------------
    snap_sums = np.empty(C, np.float64)
    fin_sums = np.empty(C, np.float64)
    d_last = None
    for c in range(N_CORES):
        for s in range(S):
            j0 = (c * S + s) * G
            snap = res.results[c][f"snap{s}"].astype(np.float64)
            fin = res.results[c][f"fin{s}"].astype(np.float64)
            snap_sums[j0:j0 + G] = snap.sum(axis=0)
            fin_sums[j0:j0 + G] = fin.sum(axis=0)
            if c == N_CORES - 1 and s == S - 1:
                d_last = fin[:, G - 1] / fin[:, G - 1].sum()

    # group 0 exactly on host: start from begin-state e_{126}
    u = np.zeros(NL, np.float64)
    u[L] = 1.0
    log_g0 = 0.0
    for t in range(CHUNK):
        u = (u @ expTz64) * np.exp(obs_pad[t].astype(np.float64))
        ssum = u.sum()
        log_g0 += np.log(ssum)
        u /= ssum

    ratios = np.log(fin_sums[1:]) - np.log(snap_sums[1:])
    logZ = (
        log_g0
        + ratios.sum() + (C - 1) * CHUNK * mu
        + np.log(np.dot(d_last, np.exp(T64[:, L + 1])))
    )

    # gold path score
    idx = np.arange(SEQ)
    real = pred.astype(np.float64)[idx, ref].sum()
    padded = np.concatenate(
        [np.array([L], ref.dtype), ref, np.array([L + 1], ref.dtype)]
    )
    real += T64[padded[:-1], padded[1:]].sum()

    return np.float32(logZ - real)


# revision 45
# speedup vs baseline: 1.3034x; 1.0330x over previous
"""CRF loss kernel for Trainium2 (8 NeuronCores).

Forward CRF recursion in linear space:
    alpha_t = (alpha_{t-1} @ expT) * exp(o_t)
Positive matrices forget initial conditions geometrically (Birkhoff
contraction; here expT entries are within e^{+-0.5} of 1, so the rate is
~5x per step). The sequence is therefore cut into C short chunks; each
chunk's chain is started from a uniform state W steps early (burn-in) so
its state direction converges to the true alpha before the chunk begins.
Per-group log-growth over the chunk is exact, directions are accurate to
~kappa^W; the scalar log-partition is stitched on the host in fp64.

Device layout: state V[128 labels (partitions), G groups (free dim)].
One matmul with the constant expT as stationary weights advances ALL G
groups one step; a DVE elementwise multiply applies the per-position
emission factors (host-gathered tiles). Emissions are pre-scaled by
exp(-mu) (mu = mean per-step log growth) so no renormalisation is ever
needed: fp32 state drift stays within e^{+-3}.

Group 0 has no real burn-in data (clamped), so its chunk is recomputed
exactly on the host in fp64 and its device result discarded.
"""

import os
import numpy as np

SEQ = 131072
L = 126          # real labels; transitions is (L+2, L+2) = (128, 128)
NL = 128
N_CORES = 8

# ---- tunables -------------------------------------------------------------
S = 4            # superchains (PSUM banks) per core
G = 512          # groups per superchain (one matmul/DVE op advances G chains)
W = 2            # burn-in steps
DMA_BATCH = 5    # steps loaded per emission DMA
# eviction-path schedule, indexed by global step id: A = DVE multiply from
# PSUM (1x); B = ACT copy to SBUF + DVE bf16 multiply (2x); C = ACT copy +
# GpSimd multiply. Spreads the per-step elementwise work over 3 engines.
PATTERN = "ABCABCABAB"
E_DT = "bfloat16"  # emission/state dtype on device: "float32" or "bfloat16"
# ---------------------------------------------------------------------------

C = N_CORES * S * G               # total groups
CHUNK = SEQ // C                  # steps per group
T = W + CHUNK                     # device steps per superchain
assert CHUNK * C == SEQ

last_exec_time_ns = None


def _build_program_raw():
    """Hand-scheduled Bass program (no Tile): per-superchain fixed paths
    s0,s1 = A (DVE multiplies straight from PSUM), s2 = B (ACT evicts to
    SBUF, DVE multiplies at 2x), s3 = C (ACT evicts, GpSimd multiplies,
    with an explicit drain before the semaphore increment for cross-engine
    write visibility). Semaphore counts are real counters, so dependencies
    propagate transitively; each op needs at most two standalone waits.
    """
    from contextlib import ExitStack
    import concourse.bacc as bacc
    import concourse.mybir as mybir

    e_dt = getattr(mybir.dt, E_DT)
    f32 = mybir.dt.float32
    assert S == 4 and W >= 2

    nc = bacc.Bacc("TRN2", target_bir_lowering=False, debug=False)
    w_d = nc.dram_tensor("w", [NL, NL], e_dt, kind="ExternalInput")
    e_ds = [nc.dram_tensor(f"e{s}", [NL, T * G], e_dt, kind="ExternalInput")
            for s in range(S)]
    snap_ds = [nc.dram_tensor(f"snap{s}", [NL, G], e_dt, kind="ExternalOutput")
               for s in range(S)]
    fin_ds = [nc.dram_tensor(f"fin{s}", [NL, G], e_dt, kind="ExternalOutput")
              for s in range(S)]

    def cnt_mm(t, s):           # matmuls run for t >= 1
        return (t - 1) * S + s + 1

    def cnt_dve(t, j):          # j = s; t=0 ops are the init copies
        return t * 4 + j + 1

    def cnt_act(t, k):          # k: 0,1,2 = evict s1,s2,s3; t >= 1
        return (t - 1) * 3 + k + 1

    QCOLS = T * G // 4
    def q_of(t):
        return min(3, ((t + 1) * G - 1) // QCOLS)

    prod_sem_of = {0: "dve", 1: "dve", 2: "dve", 3: "dve"}

    def cnt_prod(t, s):
        return cnt_dve(t, s)

    with ExitStack() as ctx:
        sems = {n: ctx.enter_context(nc.semaphore(n))
                for n in ("dma", "pe", "dve", "act", "gp", "out")}
        for q in range(4):
            for s in range(S):
                n = f"eq{q}_{s}"
                sems[n] = ctx.enter_context(nc.semaphore(n))
        w_t = ctx.enter_context(nc.sbuf_tensor("wt", [NL, NL], e_dt))
        e_b = [ctx.enter_context(nc.sbuf_tensor(f"eb{s}", [NL, T * G], e_dt))
               for s in range(S)]
        v_b = [[ctx.enter_context(
                    nc.sbuf_tensor(f"v{s}_{b}", [NL, G], e_dt))
                for b in range(2)] for s in range(S)]
        x_b = [[ctx.enter_context(
                    nc.sbuf_tensor(f"x{s}_{b}", [NL, G], e_dt))
                for b in range(2)] for s in (1, 2, 3)]
        ps_b = [[ctx.enter_context(
                    nc.psum_tensor(f"ps{s}_{b}", [NL, G], f32))
                 for b in range(2)] for s in range(S)]

        class Waiter:
            """Emit a wait only if it raises this engine's known floor."""
            def __init__(self, eng):
                self.eng = eng
                self.seen = {}

            def __call__(self, sem_name, val):
                if self.seen.get(sem_name, -1) >= val:
                    return
                self.seen[sem_name] = val
                self.eng.wait_ge(sems[sem_name], val)

        with nc.Block() as block:

            @block.sync
            def _(sync):
                wt = Waiter(sync)
                sync.dma_start(w_t[:], w_d[:]).then_inc(sems["dma"], 16)
                for q in range(4):
                    for s in range(S):
                        sl = slice(q * QCOLS, (q + 1) * QCOLS)
                        sync.dma_start(e_b[s][:, sl], e_ds[s][:, sl]) \
                            .then_inc(sems[f"eq{q}_{s}"], 16)
                for s in range(S):
                    wt(prod_sem_of[s], cnt_prod(W - 1, s))
                    sync.dma_start(snap_ds[s][:], v_b[s][(W - 1) % 2][:]) \
                        .then_inc(sems["out"], 16)
                for s in range(S):
                    wt(prod_sem_of[s], cnt_prod(T - 1, s))
                    sync.dma_start(fin_ds[s][:], v_b[s][(T - 1) % 2][:]) \
                        .then_inc(sems["out"], 16)
                sync.wait_ge(sems["out"], 16 * 2 * S)

            @block.tensor
            def _(tensor):
                wt = Waiter(tensor)
                wt("dma", 16)  # weights resident
                for t in range(1, T):
                    for s in range(S):
                        wt(prod_sem_of[s], cnt_prod(t - 1, s))
                        tensor.matmul(
                            ps_b[s][t % 2][:], w_t[:],
                            v_b[s][(t - 1) % 2][:],
                            start=True, stop=True,
                        ).then_inc(sems["pe"], 1)

            @block.vector
            def _(vector):
                wt = Waiter(vector)
                for s in range(S):
                    wt(f"eq0_{s}", 16)
                    vector.tensor_copy(
                        v_b[s][0][:], e_b[s][:, 0:G]).then_inc(sems["dve"], 1)
                for t in range(1, T):
                    if t == W + 1:
                        wt("out", 16 * S)  # snapshots shipped before reuse
                    for s in range(S):
                        wt(f"eq{q_of(t)}_{s}", 16)
                        if s >= 1:
                            wt("act", cnt_act(t, s - 1))
                            in0 = x_b[s - 1][t % 2][:]
                        else:
                            wt("pe", cnt_mm(t, s))
                            in0 = ps_b[s][t % 2][:]
                        vector.tensor_tensor(
                            v_b[s][t % 2][:], in0,
                            e_b[s][:, t * G:(t + 1) * G],
                            op=mybir.AluOpType.mult,
                        ).then_inc(sems["dve"], 1)

            @block.scalar
            def _(scalar):
                wt = Waiter(scalar)
                for t in range(1, T):
                    for k, s in ((0, 1), (1, 2), (2, 3)):
                        wt("pe", cnt_mm(t, s))
                        if t >= 3:
                            wt("dve", cnt_dve(t - 2, s))
                        scalar.activation(
                            x_b[s - 1][t % 2][:], ps_b[s][t % 2][:],
                            mybir.ActivationFunctionType.Copy,
                        ).then_inc(sems["act"], 1)

            @block.gpsimd
            def _(gpsimd):
                pass

        nc.compile()
        nc.finalize()
    return nc


def _build_program():
    import concourse.bacc as bacc
    import concourse.mybir as mybir
    from concourse.tile import TileContext

    e_dt = getattr(mybir.dt, E_DT)
    f32 = mybir.dt.float32

    nc = bacc.Bacc("TRN2", target_bir_lowering=False, debug=False)
    w_d = nc.dram_tensor("w", [NL, NL], e_dt, kind="ExternalInput")
    e_ds = [
        nc.dram_tensor(f"e{s}", [NL, T * G], e_dt, kind="ExternalInput")
        for s in range(S)
    ]
    snap_ds = [
        nc.dram_tensor(f"snap{s}", [NL, G], e_dt, kind="ExternalOutput")
        for s in range(S)
    ]
    fin_ds = [
        nc.dram_tensor(f"fin{s}", [NL, G], e_dt, kind="ExternalOutput")
        for s in range(S)
    ]

    with TileContext(nc) as tc:
        with tc.tile_pool(name="wp", bufs=1) as wp, \
             tc.tile_pool(name="ep", bufs=2) as ep, \
             tc.tile_pool(name="vp", bufs=3) as vp, \
             tc.tile_pool(name="xp", bufs=2) as xp, \
             tc.tile_pool(name="pp", bufs=2, space="PSUM") as pp:
            # Stage weights through a DVE copy: the first matmul then waits
            # only on the DVE semaphore (PE LDW allows a single sync wait).
            w_stage = wp.tile([NL, NL], e_dt, tag="wstage")
            nc.sync.dma_start(w_stage[:], w_d[:])
            w_t = wp.tile([NL, NL], e_dt, tag="wt")
            nc.vector.tensor_copy(w_t[:], w_stage[:])
            v_cur = []
            for s in range(S):
                v0 = vp.tile([NL, G], e_dt, tag=f"v{s}")
                nc.vector.memset(v0[:], 1.0)
                v_cur.append(v0)
            assert T % DMA_BATCH == 0
            e_quad = [None] * S
            for tq in range(T // DMA_BATCH):
                for s in range(S):
                    eq = ep.tile([NL, DMA_BATCH * G], e_dt, tag=f"e{s}")
                    nc.sync.dma_start(
                        eq[:],
                        e_ds[s][:, tq * DMA_BATCH * G:(tq + 1) * DMA_BATCH * G],
                    )
                    e_quad[s] = eq
                for dt in range(DMA_BATCH):
                    t = tq * DMA_BATCH + dt
                    for s in range(S):
                        ps = pp.tile([NL, G], f32, tag=f"ps{s}")
                        nc.tensor.matmul(ps[:], w_t[:], v_cur[s][:],
                                         start=True, stop=True)
                        e_ap = e_quad[s][:, dt * G:(dt + 1) * G]
                        path = PATTERN[(t * S + s) % len(PATTERN)]
                        v_n = vp.tile([NL, G], e_dt, tag=f"v{s}")
                        if path == "A":
                            nc.vector.tensor_tensor(
                                v_n[:], ps[:], e_ap,
                                op=mybir.AluOpType.mult)
                        else:
                            x = xp.tile([NL, G], e_dt, tag=f"x{s}")
                            nc.scalar.activation(
                                x[:], ps[:],
                                mybir.ActivationFunctionType.Copy)
                            engine = (nc.vector if path == "B"
                                      else nc.gpsimd)
                            engine.tensor_tensor(
                                v_n[:], x[:], e_ap,
                                op=mybir.AluOpType.mult)
                        v_cur[s] = v_n
                        if t == W - 1:
                            nc.sync.dma_start(snap_ds[s][:], v_n[:])
                        if t == T - 1:
                            nc.sync.dma_start(fin_ds[s][:], v_n[:])
    nc.compile()
    nc.finalize()
    return nc


def _profiled_run(nc, in_maps):
    """Run via PJRT with local NTFF profiling (core 0); returns (res, ns)."""
    import tempfile
    from concourse.bass_utils import run_bass_kernel_spmd
    from trn_agent_boot.trn_boot import _ntff_profile_via_ctypes

    hookf = _ntff_profile_via_ctypes("/opt/axon/libaxon_pjrt.so")
    neff_dir = tempfile.mkdtemp(prefix="crfprof_")
    exec_ns = None
    if hookf is None:
        res = run_bass_kernel_spmd(nc, in_maps, list(range(N_CORES)))
        return res, None
    with hookf(neff_dir, [0]):
        res = run_bass_kernel_spmd(nc, in_maps, list(range(N_CORES)))
    try:
        import gauge.profiler
        from concourse._compat import FishPath

        profile = gauge.profiler.Profile(
            profile_path=FishPath(neff_dir),
            kernel_dev_mode=True,
            profile_on_exit=False,
            bass_kernel=nc.m,
            offline_processing=True,
            fname="*_body*",
        )
        results = profile.to_perfetto(model_index=(0,))
        if results:
            exec_ns = results[0].exec_time_ns
            print(f"[profile] core0 exec {exec_ns} ns, "
                  f"trace: {results[0].trace_path}")
    except Exception as e:  # profiling must never break the run
        import traceback
        traceback.print_exc()
        print(f"[profile] failed: {e!r} (dir {neff_dir})")
    return res, exec_ns


def _estimate_mu(obs_pad, expTz64, n=256, skip=32):
    """Mean per-step log growth of the linear-space recursion (fp64)."""
    u = np.full(NL, 1.0 / NL, np.float64)
    logs = []
    for t in range(skip + n):
        u = (u @ expTz64) * np.exp(obs_pad[t].astype(np.float64))
        ssum = u.sum()
        logs.append(np.log(ssum))
        u /= ssum
    return float(np.mean(logs[skip:]))


def kernel(pred: np.ndarray, transitions: np.ndarray, ref: np.ndarray) -> np.ndarray:
    global last_exec_time_ns
    pred = np.asarray(pred)
    transitions = np.asarray(transitions)
    ref = np.asarray(ref)
    assert pred.shape == (SEQ, L)

    T64 = transitions.astype(np.float64)
    expTz64 = np.exp(T64)
    expTz64[:, L:] = 0.0            # dummy labels carry no mass mid-sequence

    obs_pad = np.zeros((SEQ, NL), np.float32)
    obs_pad[:, :L] = pred

    mu = _estimate_mu(obs_pad, expTz64)

    # E128T[l, p] = exp(obs[p, l] - mu), transposed, with W left-pad columns
    # replicating position 0 (burn-in clamp for the first groups).
    E128 = np.exp(obs_pad - np.float32(mu))
    E128T = np.ascontiguousarray(E128.T)                  # [128, SEQ]
    EPAD = np.concatenate(
        [np.repeat(E128T[:, :1], W, axis=1), E128T], axis=1
    )                                                     # [128, W + SEQ]

    np_e_dt = np.float32
    if E_DT == "bfloat16":
        import ml_dtypes
        np_e_dt = ml_dtypes.bfloat16
        EPAD = EPAD.astype(np_e_dt)

    # Device emission tiles: e[c][s][l, t, g] = EPAD[l, A + t + g*CHUNK],
    # A = (c*S + s)*G*CHUNK  (EPAD coords already include the +W shift).
    itemsize = EPAD.strides[1]
    e_blocks = []
    for c in range(N_CORES):
        per_s = []
        for s in range(S):
            A = (c * S + s) * G * CHUNK
            blk = np.lib.stride_tricks.as_strided(
                EPAD[:, A:],
                shape=(NL, T, G),
                strides=(EPAD.strides[0], itemsize, CHUNK * itemsize),
            )
            per_s.append(np.ascontiguousarray(blk).reshape(NL, T * G))
        e_blocks.append(per_s)

    wz32 = expTz64.astype(np_e_dt)
    if os.environ.get("CRF_TILE"):
        nc = _build_program()
    else:
        nc = _build_program_raw()

    from concourse.bass_utils import run_bass_kernel_spmd
    in_maps = [
        {"w": wz32, **{f"e{s}": e_blocks[c][s] for s in range(S)}}
        for c in range(N_CORES)
    ]
    if os.environ.get("CRF_TRACE"):
        res, last_exec_time_ns = _profiled_run(nc, in_maps)
    else:
        res = run_bass_kernel_spmd(nc, in_maps, list(range(N_CORES)))

    # --- host stitch (fp64) -------------------------------------------------
    snap_sums = np.empty(C, np.float64)
    fin_sums = np.empty(C, np.float64)
    d_last = None
    for c in range(N_CORES):
        for s in range(S):
            j0 = (c * S + s) * G
            snap = res.results[c][f"snap{s}"].astype(np.float64)
            fin = res.results[c][f"fin{s}"].astype(np.float64)
            snap_sums[j0:j0 + G] = snap.sum(axis=0)
            fin_sums[j0:j0 + G] = fin.sum(axis=0)
            if c == N_CORES - 1 and s == S - 1:
                d_last = fin[:, G - 1] / fin[:, G - 1].sum()

    # group 0 exactly on host: start from begin-state e_{126}
    u = np.zeros(NL, np.float64)
    u[L] = 1.0
    log_g0 = 0.0
    for t in range(CHUNK):
        u = (u @ expTz64) * np.exp(obs_pad[t].astype(np.float64))
        ssum = u.sum()
        log_g0 += np.log(ssum)
        u /= ssum

    ratios = np.log(fin_sums[1:]) - np.log(snap_sums[1:])
    logZ = (
        log_g0
        + ratios.sum() + (C - 1) * CHUNK * mu
        + np.log(np.dot(d_last, np.exp(T64[:, L + 1])))
    )

    # gold path score
    idx = np.arange(SEQ)
    real = pred.astype(np.float64)[idx, ref].sum()
    padded = np.concatenate(
        [np.array([L], ref.dtype), ref, np.array([L + 1], ref.dtype)]
    )
    real += T64[padded[:-1], padded[1:]].sum()

    return np.float32(logZ - real)
